# revision 21
# baseline (speedup 1.0000x reference)
"""Cross-attention (B=4, L=2048, D=1024, H=16) on 8 TRN2 NeuronCores.

Sharding: core c handles batch b = c//2 and head-group g = c%2 (8 heads,
512 projection features). Each core computes its heads' Q/K/V projections,
attention, and a partial output projection (contraction over its 512
features). Host sums the two partials per batch and adds the output bias.

Per-core layouts (host pre-arranged, matmul operands cast to bf16):
  xq/xk/xv [D=1024, L=2048]  activations transposed (contraction dim on
                             partitions for the projection matmuls), bf16
  wq/wk/wv [D=1024, F=512]   W[F,:].T  (d-major), bf16
  wo       [F=512, D=1024]   Wo[:,F].T (feat-major), bf16
  bqk      [2, 4, 128]       q/k biases reshaped for partition-dim loads
  bv       [512]             v bias (free-dim broadcast DMA)
Output: out [L=2048, D=1024] fp32 partial (x_g @ Wo[:,F].T), no bias.

On-device dataflow per core (all matmuls bf16 with fp32 psum accumulate):
  QT/KT [feat(4x128p), L] bf16 = (w-tile).T @ x-tile + bias
  V     [L(16x128p), 8*(V_h 64 | ones 64)] bf16 (ones interleaved so one
        M=128 matmul yields both attention output and softmax denominator)
  per (head, l_q chunk) unit, groups of 2 l_k tiles:
    E.T [l_k 128, 2*512] psum = KT_h_tile.T @ QT_h   (K=64)
    ACT exp(0.125 * E.T) reads both psum banks -> P.T bf16 in SBUF
        (no max subtraction: |E/8| < ~7 for these distributions)
    X'' [128, 512] psum += [V_h|ones].T @ P.T  (partitions 0:64 = X.T,
        64:128 = replicated denominator)
    DVE: rec = reciprocal(X''[64:128]); XT = X''[0:64] * rec
  The next unit's E groups are emitted interleaved with the current
  unit's X'' matmuls so ACT (the softmax bottleneck) never idles.
  out [l 128, j 512] psum = XT-tile.T @ wo, DVE copy, DMA -> DRAM
"""

from contextlib import ExitStack

import numpy as np
import ml_dtypes

import concourse.bass as bass
import concourse.tile as tile
import concourse.mybir as mybir
from concourse.bass_utils import run_bass_kernel_spmd

F32 = mybir.dt.float32
F32R = mybir.dt.float32r
BF16 = mybir.dt.bfloat16


class _TileContext(tile.TileContext):
    """TileContext whose kernel-tail drain splits its semaphore waits.

    The stock ``_drain_and_barrier`` attaches every outstanding semaphore
    wait to the single tail Drain instruction; the walrus build in this
    container rejects Drains with more than one sync wait ("Too many sync
    wait commands", CoreV3GenImpl setupSyncWait). Emit one single-wait NOP
    per outstanding proc on the SP queue ahead of the drain instead —
    program order on SP makes the bare drain equivalent.
    """

    def _drain_and_barrier(self, tick_clock, wait_clock):
        from concourse.vector_clock import ScopedClock, VectorClock

        gvec = list(tick_clock.global_clock)
        n = len(gvec)
        for p, tick in enumerate(gvec):
            if tick > 0:
                nop = self.nc.sync.nop(nofuse=True, hint=f"drainwait{p}")
                partial = [0] * n
                partial[p] = tick
                wait_clock.add_sem_waits(
                    nop.ins, ScopedClock({None: VectorClock(partial)})
                )
        self.nc.sync.drain()
        self.nc.all_engine_barrier()
        popped = self.nc._tile_sem_poison_stack.pop()
        assert popped is self._sem_poison
        self.nc.clear_and_free_semaphores(list(self.sems.allocated().values()))
        self.nc.all_engine_barrier()

def _legalize_waits(nc):
    """Split multi-wait instructions for this walrus build.

    The container's walrus rejects any instruction carrying more than one
    sync-wait command ("Too many sync wait commands"). Hoist all but the
    last wait of each instruction onto preceding NoOps on the same engine
    queue — queue program order makes this equivalent.
    """
    n = 0
    for f in nc.m.functions:
        for blk in f.blocks:
            insts = blk.instructions
            out = []
            changed = False
            for inst in insts:
                si = inst.sync_info
                if si is not None and len(si.on_wait) > 1:
                    waits = list(si.on_wait)
                    for w in waits[:-1]:
                        nop = mybir.InstNoOp(name=f"I-lw{n}")
                        n += 1
                        nop.engine = inst.engine
                        nop.sync_info = mybir.SyncInfo(on_wait=[w], on_update=[])
                        out.append(nop)
                    inst.sync_info = mybir.SyncInfo(
                        on_wait=[waits[-1]], on_update=list(si.on_update)
                    )
                    changed = True
                out.append(inst)
            if changed:
                blk.instructions = out


B, L, D, H = 4, 2048, 1024, 16
HD = D // H          # 64
NCORES = 8
HPG = 8              # heads per group (per core)
FG = HPG * HD        # 512 features per group
DT = D // 128        # 8 d-tiles
FT = FG // 128       # 4 feature tiles
LQ = L // 512        # 4 l_q chunks
LK = L // 128        # 16 l_k tiles


def _emit(ctx, tc):
    nc = tc.nc
    xq = nc.dram_tensor("xq", [D, L], BF16, kind="ExternalInput").ap()
    xk = nc.dram_tensor("xk", [D, L], BF16, kind="ExternalInput").ap()
    xv = nc.dram_tensor("xv", [D, L], BF16, kind="ExternalInput").ap()
    wq = nc.dram_tensor("wq", [D, FG], BF16, kind="ExternalInput").ap()
    wk = nc.dram_tensor("wk", [D, FG], BF16, kind="ExternalInput").ap()
    wv = nc.dram_tensor("wv", [D, FG], BF16, kind="ExternalInput").ap()
    wo = nc.dram_tensor("wo", [FG, D], BF16, kind="ExternalInput").ap()
    bqk = nc.dram_tensor("bqk", [2, FT, 128], F32, kind="ExternalInput").ap()
    bv = nc.dram_tensor("bv", [FG], F32, kind="ExternalInput").ap()
    out = nc.dram_tensor("out", [L, D], F32, kind="ExternalOutput").ap()

    singles = ctx.enter_context(tc.tile_pool(name="singles", bufs=1))
    wpool = ctx.enter_context(tc.tile_pool(name="wpool", bufs=9))
    wq_pool = ctx.enter_context(tc.tile_pool(name="wq_pool", bufs=DT))
    xpool = ctx.enter_context(tc.tile_pool(name="xpool", bufs=12))
    qt_pool = ctx.enter_context(tc.tile_pool(name="qt_pool", bufs=FT))
    kt_pool = ctx.enter_context(tc.tile_pool(name="kt_pool", bufs=FT))
    v_pool = ctx.enter_context(tc.tile_pool(name="v_pool", bufs=LK))
    xt_pool = ctx.enter_context(tc.tile_pool(name="xt_pool", bufs=FT))
    pt_pool = ctx.enter_context(tc.tile_pool(name="pt_pool", bufs=10))
    rec_pool = ctx.enter_context(tc.tile_pool(name="rec_pool", bufs=2))
    out_pool = ctx.enter_context(tc.tile_pool(name="out_pool", bufs=2))
    # 2 "xacc" bufs (X'' accumulators) + 2 "work" bufs (proj / outproj) so a
    # dripped proj/outproj psum alloc never waits on an xacc still being
    # drained by the unit tail (that wait was stalling the in-order PE queue
    # ~5us at chunk boundaries, and each stall re-throttled the PE clock)
    acc_psum = ctx.enter_context(tc.tile_pool(name="acc_psum", bufs=2, space="PSUM"))
    e_psum = ctx.enter_context(tc.tile_pool(name="e_psum", bufs=2, space="PSUM"))

    # --- critical-path DMAs first: wk + first xk quarter own the 16 queues
    # (the bias/broadcast DMAs used to go first and delayed the first matmul
    # by ~15us) ---
    wk_sb = []
    xk0 = []
    for dt_ in range(DT):
        t_ = wpool.tile([128, FG], BF16, name="w_t", tag="w")
        nc.sync.dma_start(out=t_, in_=wk[dt_ * 128 : (dt_ + 1) * 128, :])
        wk_sb.append(t_)
        t2 = xpool.tile([128, 512], BF16, name="x_t", tag="x")
        nc.sync.dma_start(out=t2, in_=xk[dt_ * 128 : (dt_ + 1) * 128, 0:512])
        xk0.append(t2)

    # --- constants / weights resident in SBUF ---
    bias_qk = singles.tile([128, 2, FT], F32, name="bias_qk")
    bqk_src = bass.AP(tensor=bqk.tensor, offset=bqk.offset, ap=[[1, 128], [128, 2 * FT]])
    nc.sync.dma_start(out=bias_qk.rearrange("p a b -> p (a b)"), in_=bqk_src)

    bv_bc = singles.tile([128, FG], F32, name="bv_bc")
    bv_src = bass.AP(tensor=bv.tensor, offset=bv.offset, ap=[[0, 128], [1, FG]])
    nc.sync.dma_start(out=bv_bc, in_=bv_src)

    # wo tiles allocated now; their DMAs are deferred into the ring prologue
    # so the attention-critical K/V/Q loads own the DMA queues first
    wo_sb = [singles.tile([128, D], BF16, name=f"wo_sb{ft}") for ft in range(FT)]

    # --- persistent activation tensors ---
    # V tiles hold [V_h (64 cols) | ones (64 cols)] per head, so a single
    # M=128 matmul per l_k tile accumulates both the attention output
    # (partitions 0:64) and the replicated softmax denominator (64:128).
    QT = [qt_pool.tile([128, L], BF16, name=f"qt{i}", tag="qt") for i in range(FT)]
    KT = [kt_pool.tile([128, L], BF16, name=f"kt{i}", tag="kt") for i in range(FT)]
    V = [v_pool.tile([128, HPG * 128], BF16, name=f"v{i}", tag="v") for i in range(LK)]
    XT = [xt_pool.tile([128, L], BF16, name=f"xt{i}", tag="xt") for i in range(FT)]
    for lt in range(LK):
        v3d = V[lt].rearrange("p (h c) -> p h c", c=128)
        nc.vector.memset(v3d[:, :, HD : 2 * HD], 1.0)

    # --- projections ---
    def load_w(wdram):
        w_sb = []
        for dt_ in range(DT):
            t_ = wpool.tile([128, FG], BF16, name="w_t", tag="w")
            nc.sync.dma_start(out=t_, in_=wdram[dt_ * 128 : (dt_ + 1) * 128, :])
            w_sb.append(t_)
        return w_sb

    def load_x_quarter(xdram, qrt):
        x_t = []
        for dt_ in range(DT):
            t_ = xpool.tile([128, 512], BF16, name="x_t", tag="x")
            nc.sync.dma_start(
                out=t_,
                in_=xdram[dt_ * 128 : (dt_ + 1) * 128, qrt * 512 : (qrt + 1) * 512],
            )
            x_t.append(t_)
        return x_t

    def proj_qk_ft(ti, out_sb, w_sb, x_t, qrt, ft):
        ps = acc_psum.tile([128, 512], F32, name="ps_proj", tag="work")
        for dt_ in range(DT):
            nc.tensor.matmul(
                ps,
                lhsT=w_sb[dt_][:, ft * 128 : (ft + 1) * 128],
                rhs=x_t[dt_],
                start=(dt_ == 0),
                stop=(dt_ == DT - 1),
            )
        nc.vector.tensor_scalar_add(
            out_sb[ft][:, qrt * 512 : (qrt + 1) * 512],
            ps,
            bias_qk[:, ti, ft : ft + 1],
        )

    def proj_v_tile(x_t, qrt, lt4):
        ps = acc_psum.tile([128, FG], F32, name="ps_projv", tag="work")
        for dt_ in range(DT):
            nc.tensor.matmul(
                ps,
                lhsT=x_t[dt_][:, lt4 * 128 : (lt4 + 1) * 128],
                rhs=wv_sb[dt_],
                start=(dt_ == 0),
                stop=(dt_ == DT - 1),
            )
        nc.vector.tensor_add(
            V[qrt * 4 + lt4].rearrange("p (h c) -> p h c", c=128)[:, :, 0:HD],
            ps.rearrange("p (h c) -> p h c", c=HD),
            bv_bc.rearrange("p (h c) -> p h c", c=HD),
        )

    # Emission order is chosen for earliest possible ring start: the ring's
    # first unit (head pair 0, l_q chunk 0) needs only KT[0] fully, QT[0]
    # chunk 0, and the V tiles in g-order. So: K projects fully (ft 0
    # first), V quarters 0/2/3 project pre-ring, and V quarter 1, Q0 ft
    # 1-3, Q chunks 1-3 and the wo loads are drip-fed into the ring with
    # deadlines. xv quarter 1 is DMA-loaded LAST among the xv quarters so
    # the xpool rotation never makes a pre-ring DMA wait on a drip task
    # that is emitted after ring start.
    for ft in range(FT):
        proj_qk_ft(1, KT, wk_sb, xk0, 0, ft)
    for qrt in range(1, LQ):
        x_t = load_x_quarter(xk, qrt)
        for ft in range(FT):
            proj_qk_ft(1, KT, wk_sb, x_t, qrt, ft)
    wv_sb = load_w(wv)
    xv_t = {qrt: load_x_quarter(xv, qrt) for qrt in (0, 2, 3)}
    # wq resident (wq_pool) so Q chunks can project mid-ring
    wq_sb = []
    for dt_ in range(DT):
        t_ = wq_pool.tile([128, FG], BF16, name=f"wq_sb{dt_}")
        nc.sync.dma_start(out=t_, in_=wq[dt_ * 128 : (dt_ + 1) * 128, :])
        wq_sb.append(t_)
    # xq0 allocated before xv q1: the xpool slots xv q1 recycles must have
    # only pre-ring or ring-start readers, or its DMA deadlocks the ring
    xq_t = {0: load_x_quarter(xq, 0)}
    xv_t[1] = load_x_quarter(xv, 1)
    for lt4 in range(4):
        proj_v_tile(xv_t[0], 0, lt4)
    for qrt in (2, 3):
        for lt4 in range(4):
            proj_v_tile(xv_t[qrt], qrt, lt4)
    proj_qk_ft(0, QT, wq_sb, xq_t[0], 0, 0)

    # drip tasks: (deadline_ring_step, task). Q0 ft 1-3 must be emitted
    # before the v1 tasks (xq0 readers gate the xv q1 DMA via pool
    # rotation), hence their early deadlines.
    pending_q = [(3 + ft, ("q0", ft)) for ft in range(1, FT)]
    pending_q += [(12 + lt4, ("v1", lt4)) for lt4 in range(4)]
    pending_q += [(44, ("x", 1))] + [(58 + 16 * ft, ("mm", 1, ft)) for ft in range(FT)]
    pending_q += [(70, ("wo", 0)), (74, ("wo", 1))]
    pending_q += [(108, ("x", 2))] + [(122 + 16 * ft, ("mm", 2, ft)) for ft in range(FT)]
    pending_q += [(134, ("wo", 2)), (138, ("wo", 3))]
    pending_q += [(172, ("x", 3))] + [(186 + 16 * ft, ("mm", 3, ft)) for ft in range(FT)]

    def emit_q_task(t):
        if t[0] == "x":
            xq_t[t[1]] = load_x_quarter(xq, t[1])
        elif t[0] == "mm":
            proj_qk_ft(0, QT, wq_sb, xq_t[t[1]], t[1], t[2])
        elif t[0] == "q0":
            proj_qk_ft(0, QT, wq_sb, xq_t[0], 0, t[1])
        elif t[0] == "v1":
            proj_v_tile(xv_t[1], 1, t[1])
        else:
            ft = t[1]
            nc.sync.dma_start(out=wo_sb[ft], in_=wo[ft * 128 : (ft + 1) * 128, :])

    # --- attention + output projection, software-pipelined ---
    # Heads are processed in even/odd pairs (p -> heads 2p, 2p+1, same KT/QT
    # partition tile, partitions 0:64 and 64:128). A group is one l_k tile
    # of one (pair, l_q chunk) unit: the two heads' E.T matmuls hit
    # disjoint PE row groups, so they run concurrently and hide each
    # other's weight loads; they fill the two banks of a [128, 1024] psum
    # tile that a single ACT exp drains to bf16 P.T. Each head's X''
    # matmul ([V_h|ones].T @ P.T) accumulates into its own [128, 512] psum
    # tile (0:64 = X.T, 64:128 = replicated denominator). The group stream
    # runs through an 8-group software-pipeline ring (E of group j+8 is
    # emitted next to X'' of group j) so ACT, the softmax bottleneck,
    # never waits for PE. After the 8 heads of an l_q chunk finish, its
    # output-projection rows are emitted.
    units = [(p, lq) for lq in range(LQ) for p in range(HPG // 2)]
    NU = len(units)
    LOOKAHEAD = 8

    def emit_e_group(j):
        u, g = divmod(j, LK)
        p, lq = units[u]
        ep = e_psum.tile([128, 1024], F32, name="ep", tag="ep")
        for i in range(2):
            po = i * 64
            nc.tensor.matmul(
                ep[:, i * 512 : (i + 1) * 512],
                lhsT=KT[p][po : po + 64, g * 128 : (g + 1) * 128],
                rhs=QT[p][po : po + 64, lq * 512 : (lq + 1) * 512],
                tile_position=(po, 0),
                skip_group_check=True,
            )
        pt = pt_pool.tile([128, 2, 512], BF16, name="pt", tag="pt")
        nc.scalar.activation(
            out=pt,
            in_=ep.rearrange("p (a b) -> p a b", a=2),
            func=mybir.ActivationFunctionType.Exp,
            scale=0.125,
        )
        return pt

    def emit_x_group(j, xaccs, pt):
        u, g = divmod(j, LK)
        p, lq = units[u]
        for i in range(2):
            h = 2 * p + i
            nc.tensor.matmul(
                xaccs[i],
                lhsT=V[g][:, h * 128 : (h + 1) * 128],
                rhs=pt[:, i, :],
                start=(g == 0),
                stop=(g == LK - 1),
                skip_group_check=True,
            )

    def emit_tail(u, xaccs):
        p, lq = units[u]
        # Drain each xacc with two partition-shifted PSUM copies (legal for
        # PSUM operands) packing both heads into single [128, 512] tiles:
        # xsb = [X_h0 | X_h1], den = [d_h0 | d_h1]. The psum banks recycle
        # after ~4 cheap copies instead of after the 3.3us reciprocal, and
        # ONE reciprocal + ONE multiply then cover both heads (DVE time
        # scales with columns, not partitions).
        xsb = rec_pool.tile([128, 512], BF16, name="xsb", tag="xsb", bufs=1)
        den = rec_pool.tile([128, 512], F32, name="den", tag="den", bufs=1)
        nc.vector.tensor_copy(xsb[0:64, :], xaccs[0][0:64, :])
        nc.vector.tensor_copy(den[0:64, :], xaccs[0][64:128, :])
        nc.vector.tensor_copy(xsb[64:128, :], xaccs[1][0:64, :])
        nc.vector.tensor_copy(den[64:128, :], xaccs[1][64:128, :])
        rec = rec_pool.tile([128, 512], F32, name="rec", tag="rec", bufs=1)
        nc.vector.reciprocal(rec, den)
        nc.vector.tensor_mul(
            XT[p][:, lq * 512 : (lq + 1) * 512], xsb, rec
        )

    def emit_outproj_tile(lt, jt):
        ps = acc_psum.tile([128, 512], F32, name="ps_out", tag="work")
        for ft_ in range(FT):
            nc.tensor.matmul(
                ps,
                lhsT=XT[ft_][:, lt * 128 : (lt + 1) * 128],
                rhs=wo_sb[ft_][:, jt * 512 : (jt + 1) * 512],
                start=(ft_ == 0),
                stop=(ft_ == FT - 1),
            )
        osb = out_pool.tile([128, 512], F32, name="osb", tag="osb")
        nc.vector.tensor_copy(osb, ps)
        nc.sync.dma_start(
            out=out[lt * 128 : (lt + 1) * 128, jt * 512 : (jt + 1) * 512],
            in_=osb,
        )

    NJ = NU * LK
    pts = {}
    xaccs = None
    pending_out = []
    pending_q.sort(key=lambda dt_t: dt_t[0])

    def drip(j, n):
        # emit up to n units of deferred work (outproj tile = 4 MMs,
        # q-task = up to 8 MMs) into the PE queue at this point
        done = 0
        while done < n:
            if pending_out and pending_out[0][2] <= j:
                lt, jt, _ = pending_out.pop(0)
                emit_outproj_tile(lt, jt)
            elif pending_q:
                emit_q_task(pending_q.pop(0)[1])
            else:
                break
            done += 1

    for j in range(NJ + LOOKAHEAD):
        # deadline catch-up: emit producer tasks (V quarter 1, Q0 ft 1-3,
        # Q chunks, wo loads) before the ring steps that consume them
        while pending_q and pending_q[0][0] <= j + 8:
            emit_q_task(pending_q.pop(0)[1])
        if j < NJ:
            pts[j] = emit_e_group(j)
        jx = j - LOOKAHEAD
        if 0 <= jx < NJ:
            u, g = divmod(jx, LK)
            if g <= 1:
                # the first two X'' groups of a unit wait for the previous
                # unit's xacc psum banks, which free only after the tail's
                # drain copies (~1.5-3us): inject drip work BEFORE them so
                # the in-order PE queue has useful matmuls to chew on and
                # the PE-idle gap stays under the ~3.4us HAM re-throttle
                # window
                drip(j, 1)
            if g == 0:
                xaccs = [
                    acc_psum.tile([128, 512], F32, name=f"xacc{i}", tag="xacc")
                    for i in range(2)
                ]
            emit_x_group(jx, xaccs, pts.pop(jx))
            if g == LK - 1:
                emit_tail(u, xaccs)
                p, lq = units[u]
                if p == HPG // 2 - 1:
                    # release outproj only 12 ring steps after the chunk's
                    # tails are emitted: their XT inputs come from the DVE
                    # tail (recip+mul), and an outproj matmul queued too
                    # eagerly stalls the in-order PE queue on that latency
                    pending_out.extend(
                        (lt, jt, j + 12)
                        for lt in range(lq * 4, (lq + 1) * 4)
                        for jt in range(2)
                    )
                drip(j, 1)
        # steady-state pacing away from unit boundaries
        if jx < 0 or (jx % LK) not in (0, 1, LK - 1):
            if pending_q and j % 5 == 1:
                emit_q_task(pending_q.pop(0)[1])
            if pending_out and j % 3 == 0 and pending_out[0][2] <= j:
                lt, jt, _ = pending_out.pop(0)
                emit_outproj_tile(lt, jt)
    while pending_q:
        emit_q_task(pending_q.pop(0)[1])
    while pending_out:
        lt, jt, _ = pending_out.pop(0)
        emit_outproj_tile(lt, jt)


def build_program():
    nc = bass.Bass("TRN2", target_bir_lowering=False, debug=False, num_devices=NCORES)
    with _TileContext(nc) as tc:
        with ExitStack() as ctx:
            _emit(ctx, tc)
    return nc


def make_in_maps(query, key, value, Wq, bq, Wk, bk, Wv, bv, Wo, bo):
    query = np.asarray(query, np.float32)
    key = np.asarray(key, np.float32)
    value = np.asarray(value, np.float32)
    xqs = [np.ascontiguousarray(query[b].T).astype(ml_dtypes.bfloat16) for b in range(B)]
    xks = [np.ascontiguousarray(key[b].T).astype(ml_dtypes.bfloat16) for b in range(B)]
    xvs = [np.ascontiguousarray(value[b].T).astype(ml_dtypes.bfloat16) for b in range(B)]
    in_maps = []
    for c in range(NCORES):
        b, g = divmod(c, 2)
        fs = slice(g * FG, (g + 1) * FG)
        in_maps.append(
            {
                "xq": xqs[b],
                "xk": xks[b],
                "xv": xvs[b],
                "wq": np.ascontiguousarray(np.asarray(Wq, np.float32)[fs, :].T).astype(ml_dtypes.bfloat16),
                "wk": np.ascontiguousarray(np.asarray(Wk, np.float32)[fs, :].T).astype(ml_dtypes.bfloat16),
                "wv": np.ascontiguousarray(np.asarray(Wv, np.float32)[fs, :].T).astype(ml_dtypes.bfloat16),
                "wo": np.ascontiguousarray(
                    np.asarray(Wo, np.float32)[:, fs].T
                ).astype(ml_dtypes.bfloat16),
                "bqk": np.stack(
                    [
                        np.asarray(bq, np.float32)[fs].reshape(FT, 128),
                        np.asarray(bk, np.float32)[fs].reshape(FT, 128),
                    ]
                ),
                "bv": np.ascontiguousarray(np.asarray(bv, np.float32)[fs]),
            }
        )
    return in_maps


def _enable_ldw_opt():
    """Flip walrus's --enable-ldw-opt to true: it hides LDWEIGHTS behind
    matmul streaming (background weight buffer), which is worth ~90ns on
    every matmul here."""
    import concourse.bass_utils as _bu

    if getattr(_bu, "_ldw_patched", False):
        return
    _orig = _bu.run_command

    def _patched(argv, **kwargs):
        argv = [
            a.replace("--enable-ldw-opt=false", "--enable-ldw-opt=true")
            if isinstance(a, str)
            else a
            for a in argv
        ]
        return _orig(argv, **kwargs)

    _bu.run_command = _patched
    _bu._ldw_patched = True


def kernel(query, key, value, Wq, bq, Wk, bk, Wv, bv, Wo, bo, _trace=False):
    nc = build_program()
    _legalize_waits(nc)
    in_maps = make_in_maps(query, key, value, Wq, bq, Wk, bk, Wv, bv, Wo, bo)
    try:
        res = run_bass_kernel_spmd(
            nc, in_maps, core_ids=list(range(NCORES)), trace=_trace
        )
    except ModuleNotFoundError:
        res = run_bass_kernel_spmd(nc, in_maps, core_ids=list(range(NCORES)))
    full = np.empty((B, L, D), np.float32)
    bo32 = np.asarray(bo, np.float32)
    for b in range(B):
        full[b] = res.results[2 * b]["out"] + res.results[2 * b + 1]["out"] + bo32
    if _trace:
        kernel._last_trace = res
    return full



# revision 22
# speedup vs baseline: 1.0268x; 1.0268x over previous
"""Cross-attention (B=4, L=2048, D=1024, H=16) on 8 TRN2 NeuronCores.

Sharding: core c handles batch b = c//2 and head-group g = c%2 (8 heads,
512 projection features). Each core computes its heads' Q/K/V projections,
attention, and a partial output projection (contraction over its 512
features). Host sums the two partials per batch and adds the output bias.

Per-core layouts (host pre-arranged, matmul operands cast to bf16):
  xq/xk/xv [D=1024, L=2048]  activations transposed (contraction dim on
                             partitions for the projection matmuls), bf16
  wq/wk/wv [D=1024, F=512]   W[F,:].T  (d-major), bf16
  wo       [F=512, D=1024]   Wo[:,F].T (feat-major), bf16
  bqk      [2, 4, 128]       q/k biases reshaped for partition-dim loads
  bv       [512]             v bias (free-dim broadcast DMA)
Output: out [L=2048, D=1024] fp32 partial (x_g @ Wo[:,F].T), no bias.

On-device dataflow per core (all matmuls bf16 with fp32 psum accumulate):
  QT/KT [feat(4x128p), L] bf16 = (w-tile).T @ x-tile + bias
  V     [L(16x128p), 8*(V_h 64 | ones 64)] bf16 (ones interleaved so one
        M=128 matmul yields both attention output and softmax denominator)
  per (head, l_q chunk) unit, groups of 2 l_k tiles:
    E.T [l_k 128, 2*512] psum = KT_h_tile.T @ QT_h   (K=64)
    ACT exp(0.125 * E.T) reads both psum banks -> P.T bf16 in SBUF
        (no max subtraction: |E/8| < ~7 for these distributions)
    X'' [128, 512] psum += [V_h|ones].T @ P.T  (partitions 0:64 = X.T,
        64:128 = replicated denominator)
    DVE: rec = reciprocal(X''[64:128]); XT = X''[0:64] * rec
  The next unit's E groups are emitted interleaved with the current
  unit's X'' matmuls so ACT (the softmax bottleneck) never idles.
  out [l 128, j 512] psum = XT-tile.T @ wo, DVE copy, DMA -> DRAM
"""

from contextlib import ExitStack

import numpy as np
import ml_dtypes

import concourse.bass as bass
import concourse.tile as tile
import concourse.mybir as mybir
from concourse.bass_utils import run_bass_kernel_spmd

F32 = mybir.dt.float32
F32R = mybir.dt.float32r
BF16 = mybir.dt.bfloat16


class _TileContext(tile.TileContext):
    """TileContext whose kernel-tail drain splits its semaphore waits.

    The stock ``_drain_and_barrier`` attaches every outstanding semaphore
    wait to the single tail Drain instruction; the walrus build in this
    container rejects Drains with more than one sync wait ("Too many sync
    wait commands", CoreV3GenImpl setupSyncWait). Emit one single-wait NOP
    per outstanding proc on the SP queue ahead of the drain instead —
    program order on SP makes the bare drain equivalent.
    """

    def _drain_and_barrier(self, tick_clock, wait_clock):
        from concourse.vector_clock import ScopedClock, VectorClock

        gvec = list(tick_clock.global_clock)
        n = len(gvec)
        for p, tick in enumerate(gvec):
            if tick > 0:
                nop = self.nc.sync.nop(nofuse=True, hint=f"drainwait{p}")
                partial = [0] * n
                partial[p] = tick
                wait_clock.add_sem_waits(
                    nop.ins, ScopedClock({None: VectorClock(partial)})
                )
        self.nc.sync.drain()
        self.nc.all_engine_barrier()
        popped = self.nc._tile_sem_poison_stack.pop()
        assert popped is self._sem_poison
        self.nc.clear_and_free_semaphores(list(self.sems.allocated().values()))
        self.nc.all_engine_barrier()

def _legalize_waits(nc):
    """Split multi-wait instructions for this walrus build.

    The container's walrus rejects any instruction carrying more than one
    sync-wait command ("Too many sync wait commands"). Hoist all but the
    last wait of each instruction onto preceding NoOps on the same engine
    queue — queue program order makes this equivalent.
    """
    n = 0
    for f in nc.m.functions:
        for blk in f.blocks:
            insts = blk.instructions
            out = []
            changed = False
            for inst in insts:
                si = inst.sync_info
                if si is not None and len(si.on_wait) > 1:
                    waits = list(si.on_wait)
                    for w in waits[:-1]:
                        nop = mybir.InstNoOp(name=f"I-lw{n}")
                        n += 1
                        nop.engine = inst.engine
                        nop.sync_info = mybir.SyncInfo(on_wait=[w], on_update=[])
                        out.append(nop)
                    inst.sync_info = mybir.SyncInfo(
                        on_wait=[waits[-1]], on_update=list(si.on_update)
                    )
                    changed = True
                out.append(inst)
            if changed:
                blk.instructions = out


B, L, D, H = 4, 2048, 1024, 16
HD = D // H          # 64
NCORES = 8
HPG = 8              # heads per group (per core)
FG = HPG * HD        # 512 features per group
DT = D // 128        # 8 d-tiles
FT = FG // 128       # 4 feature tiles
LQ = L // 512        # 4 l_q chunks
LK = L // 128        # 16 l_k tiles


def _emit(ctx, tc):
    nc = tc.nc
    xq = nc.dram_tensor("xq", [D, L], BF16, kind="ExternalInput").ap()
    xk = nc.dram_tensor("xk", [D, L], BF16, kind="ExternalInput").ap()
    xv = nc.dram_tensor("xv", [D, L], BF16, kind="ExternalInput").ap()
    wq = nc.dram_tensor("wq", [D, FG], BF16, kind="ExternalInput").ap()
    wk = nc.dram_tensor("wk", [D, FG], BF16, kind="ExternalInput").ap()
    wv = nc.dram_tensor("wv", [D, FG], BF16, kind="ExternalInput").ap()
    wo = nc.dram_tensor("wo", [FG, D], BF16, kind="ExternalInput").ap()
    bqk = nc.dram_tensor("bqk", [2, FT, 128], F32, kind="ExternalInput").ap()
    bv = nc.dram_tensor("bv", [FG], F32, kind="ExternalInput").ap()
    out = nc.dram_tensor("out", [L, D], F32, kind="ExternalOutput").ap()

    singles = ctx.enter_context(tc.tile_pool(name="singles", bufs=1))
    wpool = ctx.enter_context(tc.tile_pool(name="wpool", bufs=9))
    wq_pool = ctx.enter_context(tc.tile_pool(name="wq_pool", bufs=DT))
    xpool = ctx.enter_context(tc.tile_pool(name="xpool", bufs=12))
    qt_pool = ctx.enter_context(tc.tile_pool(name="qt_pool", bufs=FT))
    kt_pool = ctx.enter_context(tc.tile_pool(name="kt_pool", bufs=FT))
    v_pool = ctx.enter_context(tc.tile_pool(name="v_pool", bufs=LK))
    xt_pool = ctx.enter_context(tc.tile_pool(name="xt_pool", bufs=FT))
    pt_pool = ctx.enter_context(tc.tile_pool(name="pt_pool", bufs=10))
    rec_pool = ctx.enter_context(tc.tile_pool(name="rec_pool", bufs=2))
    out_pool = ctx.enter_context(tc.tile_pool(name="out_pool", bufs=2))
    # 2 "xacc" bufs (X'' accumulators) + 2 "work" bufs (proj / outproj) so a
    # dripped proj/outproj psum alloc never waits on an xacc still being
    # drained by the unit tail (that wait was stalling the in-order PE queue
    # ~5us at chunk boundaries, and each stall re-throttled the PE clock)
    acc_psum = ctx.enter_context(tc.tile_pool(name="acc_psum", bufs=2, space="PSUM"))
    e_psum = ctx.enter_context(tc.tile_pool(name="e_psum", bufs=2, space="PSUM"))

    # --- critical-path DMAs first: wk + first xk quarter own the 16 queues
    # (the bias/broadcast DMAs used to go first and delayed the first matmul
    # by ~15us) ---
    wk_sb = []
    xk0 = []
    for dt_ in range(DT):
        t_ = wpool.tile([128, FG], BF16, name="w_t", tag="w")
        nc.sync.dma_start(out=t_, in_=wk[dt_ * 128 : (dt_ + 1) * 128, :])
        wk_sb.append(t_)
        t2 = xpool.tile([128, 512], BF16, name="x_t", tag="x")
        nc.sync.dma_start(out=t2, in_=xk[dt_ * 128 : (dt_ + 1) * 128, 0:512])
        xk0.append(t2)

    # --- constants / weights resident in SBUF ---
    bias_qk = singles.tile([128, 2, FT], F32, name="bias_qk")
    bqk_src = bass.AP(tensor=bqk.tensor, offset=bqk.offset, ap=[[1, 128], [128, 2 * FT]])
    nc.sync.dma_start(out=bias_qk.rearrange("p a b -> p (a b)"), in_=bqk_src)

    bv_bc = singles.tile([128, FG], F32, name="bv_bc")
    bv_src = bass.AP(tensor=bv.tensor, offset=bv.offset, ap=[[0, 128], [1, FG]])
    nc.sync.dma_start(out=bv_bc, in_=bv_src)

    # wo tiles allocated now; their DMAs are deferred into the ring prologue
    # so the attention-critical K/V/Q loads own the DMA queues first
    wo_sb = [singles.tile([128, D], BF16, name=f"wo_sb{ft}") for ft in range(FT)]

    # --- persistent activation tensors ---
    # V tiles hold [V_h (64 cols) | ones (64 cols)] per head, so a single
    # M=128 matmul per l_k tile accumulates both the attention output
    # (partitions 0:64) and the replicated softmax denominator (64:128).
    QT = [qt_pool.tile([128, L], BF16, name=f"qt{i}", tag="qt") for i in range(FT)]
    KT = [kt_pool.tile([128, L], BF16, name=f"kt{i}", tag="kt") for i in range(FT)]
    V = [v_pool.tile([128, HPG * 128], BF16, name=f"v{i}", tag="v") for i in range(LK)]
    XT = [xt_pool.tile([128, L], BF16, name=f"xt{i}", tag="xt") for i in range(FT)]
    for lt in range(LK):
        v3d = V[lt].rearrange("p (h c) -> p h c", c=128)
        nc.vector.memset(v3d[:, :, HD : 2 * HD], 1.0)

    # --- projections ---
    def load_w(wdram):
        w_sb = []
        for dt_ in range(DT):
            t_ = wpool.tile([128, FG], BF16, name="w_t", tag="w")
            nc.sync.dma_start(out=t_, in_=wdram[dt_ * 128 : (dt_ + 1) * 128, :])
            w_sb.append(t_)
        return w_sb

    def load_x_quarter(xdram, qrt):
        x_t = []
        for dt_ in range(DT):
            t_ = xpool.tile([128, 512], BF16, name="x_t", tag="x")
            nc.sync.dma_start(
                out=t_,
                in_=xdram[dt_ * 128 : (dt_ + 1) * 128, qrt * 512 : (qrt + 1) * 512],
            )
            x_t.append(t_)
        return x_t

    def proj_qk_ft(ti, out_sb, w_sb, x_t, qrt, ft):
        ps = acc_psum.tile([128, 512], F32, name="ps_proj", tag="work")
        for dt_ in range(DT):
            nc.tensor.matmul(
                ps,
                lhsT=w_sb[dt_][:, ft * 128 : (ft + 1) * 128],
                rhs=x_t[dt_],
                start=(dt_ == 0),
                stop=(dt_ == DT - 1),
            )
        nc.vector.tensor_scalar_add(
            out_sb[ft][:, qrt * 512 : (qrt + 1) * 512],
            ps,
            bias_qk[:, ti, ft : ft + 1],
        )

    def proj_v_tile(x_t, qrt, lt4):
        ps = acc_psum.tile([128, FG], F32, name="ps_projv", tag="work")
        for dt_ in range(DT):
            nc.tensor.matmul(
                ps,
                lhsT=x_t[dt_][:, lt4 * 128 : (lt4 + 1) * 128],
                rhs=wv_sb[dt_],
                start=(dt_ == 0),
                stop=(dt_ == DT - 1),
            )
        nc.vector.tensor_add(
            V[qrt * 4 + lt4].rearrange("p (h c) -> p h c", c=128)[:, :, 0:HD],
            ps.rearrange("p (h c) -> p h c", c=HD),
            bv_bc.rearrange("p (h c) -> p h c", c=HD),
        )

    # Emission order is chosen for earliest possible ring start: the ring's
    # first unit (head pair 0, l_q chunk 0) needs only KT[0] fully, QT[0]
    # chunk 0, and the V tiles in g-order. So: K projects fully (ft 0
    # first), V quarters 0/2/3 project pre-ring, and V quarter 1, Q0 ft
    # 1-3, Q chunks 1-3 and the wo loads are drip-fed into the ring with
    # deadlines. xv quarter 1 is DMA-loaded LAST among the xv quarters so
    # the xpool rotation never makes a pre-ring DMA wait on a drip task
    # that is emitted after ring start.
    for ft in range(FT):
        proj_qk_ft(1, KT, wk_sb, xk0, 0, ft)
    for qrt in range(1, LQ):
        x_t = load_x_quarter(xk, qrt)
        for ft in range(FT):
            proj_qk_ft(1, KT, wk_sb, x_t, qrt, ft)
    wv_sb = load_w(wv)
    # x loads in ring-consumption order (xv0, xq0, xv1, xv2, xv3): the
    # xpool rotation makes each load's DMA wait on the readers of the tile
    # 12 allocations back, so loads must be ordered such that those readers
    # are always emitted before the load's own consumers
    xv_t = {0: load_x_quarter(xv, 0)}
    # wq resident (wq_pool) so Q chunks can project mid-ring
    wq_sb = []
    for dt_ in range(DT):
        t_ = wq_pool.tile([128, FG], BF16, name=f"wq_sb{dt_}")
        nc.sync.dma_start(out=t_, in_=wq[dt_ * 128 : (dt_ + 1) * 128, :])
        wq_sb.append(t_)
    xq_t = {0: load_x_quarter(xq, 0)}
    for qrt in (1, 2, 3):
        xv_t[qrt] = load_x_quarter(xv, qrt)
    for lt4 in range(4):
        proj_v_tile(xv_t[0], 0, lt4)
    proj_qk_ft(0, QT, wq_sb, xq_t[0], 0, 0)

    # drip tasks: (deadline_ring_step, task). Q0 ft 1-3 must be emitted
    # before the v-quarter tasks (xq0 readers gate the xv q1 DMA via pool
    # rotation), hence their early deadlines.
    pending_q = [(3 + ft, ("q0", ft)) for ft in range(1, FT)]
    pending_q += [(12 + lt4, ("v", 1, lt4)) for lt4 in range(4)]
    pending_q += [(16 + lt4, ("v", 2, lt4)) for lt4 in range(4)]
    pending_q += [(20 + lt4, ("v", 3, lt4)) for lt4 in range(4)]
    pending_q += [(44, ("x", 1))] + [(58 + 16 * ft, ("mm", 1, ft)) for ft in range(FT)]
    pending_q += [(70, ("wo", 0)), (74, ("wo", 1))]
    pending_q += [(108, ("x", 2))] + [(122 + 16 * ft, ("mm", 2, ft)) for ft in range(FT)]
    pending_q += [(134, ("wo", 2)), (138, ("wo", 3))]
    pending_q += [(172, ("x", 3))] + [(186 + 16 * ft, ("mm", 3, ft)) for ft in range(FT)]

    def emit_q_task(t):
        if t[0] == "x":
            xq_t[t[1]] = load_x_quarter(xq, t[1])
        elif t[0] == "mm":
            proj_qk_ft(0, QT, wq_sb, xq_t[t[1]], t[1], t[2])
        elif t[0] == "q0":
            proj_qk_ft(0, QT, wq_sb, xq_t[0], 0, t[1])
        elif t[0] == "v":
            proj_v_tile(xv_t[t[1]], t[1], t[2])
        else:
            ft = t[1]
            nc.sync.dma_start(out=wo_sb[ft], in_=wo[ft * 128 : (ft + 1) * 128, :])

    # --- attention + output projection, software-pipelined ---
    # Heads are processed in even/odd pairs (p -> heads 2p, 2p+1, same KT/QT
    # partition tile, partitions 0:64 and 64:128). A group is one l_k tile
    # of one (pair, l_q chunk) unit: the two heads' E.T matmuls hit
    # disjoint PE row groups, so they run concurrently and hide each
    # other's weight loads; they fill the two banks of a [128, 1024] psum
    # tile that a single ACT exp drains to bf16 P.T. Each head's X''
    # matmul ([V_h|ones].T @ P.T) accumulates into its own [128, 512] psum
    # tile (0:64 = X.T, 64:128 = replicated denominator). The group stream
    # runs through an 8-group software-pipeline ring (E of group j+8 is
    # emitted next to X'' of group j) so ACT, the softmax bottleneck,
    # never waits for PE. After the 8 heads of an l_q chunk finish, its
    # output-projection rows are emitted.
    units = [(p, lq) for lq in range(LQ) for p in range(HPG // 2)]
    NU = len(units)
    LOOKAHEAD = 8

    def emit_e_group(j):
        u, g = divmod(j, LK)
        p, lq = units[u]
        ep = e_psum.tile([128, 1024], F32, name="ep", tag="ep")
        for i in range(2):
            po = i * 64
            nc.tensor.matmul(
                ep[:, i * 512 : (i + 1) * 512],
                lhsT=KT[p][po : po + 64, g * 128 : (g + 1) * 128],
                rhs=QT[p][po : po + 64, lq * 512 : (lq + 1) * 512],
                tile_position=(po, 0),
                skip_group_check=True,
            )
        pt = pt_pool.tile([128, 2, 512], BF16, name="pt", tag="pt")
        nc.scalar.activation(
            out=pt,
            in_=ep.rearrange("p (a b) -> p a b", a=2),
            func=mybir.ActivationFunctionType.Exp,
            scale=0.125,
        )
        return pt

    def emit_x_group(j, xaccs, pt):
        u, g = divmod(j, LK)
        p, lq = units[u]
        for i in range(2):
            h = 2 * p + i
            nc.tensor.matmul(
                xaccs[i],
                lhsT=V[g][:, h * 128 : (h + 1) * 128],
                rhs=pt[:, i, :],
                start=(g == 0),
                stop=(g == LK - 1),
                skip_group_check=True,
            )

    def emit_tail(u, xaccs):
        p, lq = units[u]
        # Drain each xacc with two partition-shifted PSUM copies (legal for
        # PSUM operands) packing both heads into single [128, 512] tiles:
        # xsb = [X_h0 | X_h1], den = [d_h0 | d_h1]. The psum banks recycle
        # after ~4 cheap copies instead of after the 3.3us reciprocal, and
        # ONE reciprocal + ONE multiply then cover both heads (DVE time
        # scales with columns, not partitions).
        xsb = rec_pool.tile([128, 512], BF16, name="xsb", tag="xsb", bufs=1)
        den = rec_pool.tile([128, 512], F32, name="den", tag="den", bufs=1)
        nc.vector.tensor_copy(xsb[0:64, :], xaccs[0][0:64, :])
        nc.vector.tensor_copy(den[0:64, :], xaccs[0][64:128, :])
        nc.vector.tensor_copy(xsb[64:128, :], xaccs[1][0:64, :])
        nc.vector.tensor_copy(den[64:128, :], xaccs[1][64:128, :])
        rec = rec_pool.tile([128, 512], F32, name="rec", tag="rec", bufs=1)
        nc.vector.reciprocal(rec, den)
        nc.vector.tensor_mul(
            XT[p][:, lq * 512 : (lq + 1) * 512], xsb, rec
        )

    def emit_outproj_tile(lt, jt):
        ps = acc_psum.tile([128, 512], F32, name="ps_out", tag="work")
        for ft_ in range(FT):
            nc.tensor.matmul(
                ps,
                lhsT=XT[ft_][:, lt * 128 : (lt + 1) * 128],
                rhs=wo_sb[ft_][:, jt * 512 : (jt + 1) * 512],
                start=(ft_ == 0),
                stop=(ft_ == FT - 1),
            )
        osb = out_pool.tile([128, 512], F32, name="osb", tag="osb")
        nc.vector.tensor_copy(osb, ps)
        nc.sync.dma_start(
            out=out[lt * 128 : (lt + 1) * 128, jt * 512 : (jt + 1) * 512],
            in_=osb,
        )

    NJ = NU * LK
    pts = {}
    xaccs = None
    pending_out = []
    pending_q.sort(key=lambda dt_t: dt_t[0])

    def drip(j, n):
        # emit up to n units of deferred work (outproj tile = 4 MMs,
        # q-task = up to 8 MMs) into the PE queue at this point
        done = 0
        while done < n:
            if pending_out and pending_out[0][2] <= j:
                lt, jt, _ = pending_out.pop(0)
                emit_outproj_tile(lt, jt)
            elif pending_q:
                emit_q_task(pending_q.pop(0)[1])
            else:
                break
            done += 1

    for j in range(NJ + LOOKAHEAD):
        # deadline catch-up: emit producer tasks (V quarter 1, Q0 ft 1-3,
        # Q chunks, wo loads) before the ring steps that consume them
        while pending_q and pending_q[0][0] <= j + 8:
            emit_q_task(pending_q.pop(0)[1])
        if j < NJ:
            pts[j] = emit_e_group(j)
        jx = j - LOOKAHEAD
        if 0 <= jx < NJ:
            u, g = divmod(jx, LK)
            if g <= 1:
                # the first two X'' groups of a unit wait for the previous
                # unit's xacc psum banks, which free only after the tail's
                # drain copies (~1.5-3us): inject drip work BEFORE them so
                # the in-order PE queue has useful matmuls to chew on and
                # the PE-idle gap stays under the ~3.4us HAM re-throttle
                # window
                drip(j, 1)
            if g == 0:
                xaccs = [
                    acc_psum.tile([128, 512], F32, name=f"xacc{i}", tag="xacc")
                    for i in range(2)
                ]
            emit_x_group(jx, xaccs, pts.pop(jx))
            if g == LK - 1:
                emit_tail(u, xaccs)
                p, lq = units[u]
                if p == HPG // 2 - 1:
                    # release outproj only 12 ring steps after the chunk's
                    # tails are emitted: their XT inputs come from the DVE
                    # tail (recip+mul), and an outproj matmul queued too
                    # eagerly stalls the in-order PE queue on that latency
                    pending_out.extend(
                        (lt, jt, j + 12)
                        for lt in range(lq * 4, (lq + 1) * 4)
                        for jt in range(2)
                    )
                drip(j, 1)
        # steady-state pacing away from unit boundaries
        if jx < 0 or (jx % LK) not in (0, 1, LK - 1):
            if pending_q and j % 5 == 1:
                emit_q_task(pending_q.pop(0)[1])
            if pending_out and j % 3 == 0 and pending_out[0][2] <= j:
                lt, jt, _ = pending_out.pop(0)
                emit_outproj_tile(lt, jt)
    while pending_q:
        emit_q_task(pending_q.pop(0)[1])
    while pending_out:
        lt, jt, _ = pending_out.pop(0)
        emit_outproj_tile(lt, jt)


def build_program():
    nc = bass.Bass("TRN2", target_bir_lowering=False, debug=False, num_devices=NCORES)
    with _TileContext(nc) as tc:
        with ExitStack() as ctx:
            _emit(ctx, tc)
    return nc


def make_in_maps(query, key, value, Wq, bq, Wk, bk, Wv, bv, Wo, bo):
    query = np.asarray(query, np.float32)
    key = np.asarray(key, np.float32)
    value = np.asarray(value, np.float32)
    xqs = [np.ascontiguousarray(query[b].T).astype(ml_dtypes.bfloat16) for b in range(B)]
    xks = [np.ascontiguousarray(key[b].T).astype(ml_dtypes.bfloat16) for b in range(B)]
    xvs = [np.ascontiguousarray(value[b].T).astype(ml_dtypes.bfloat16) for b in range(B)]
    in_maps = []
    for c in range(NCORES):
        b, g = divmod(c, 2)
        fs = slice(g * FG, (g + 1) * FG)
        in_maps.append(
            {
                "xq": xqs[b],
                "xk": xks[b],
                "xv": xvs[b],
                "wq": np.ascontiguousarray(np.asarray(Wq, np.float32)[fs, :].T).astype(ml_dtypes.bfloat16),
                "wk": np.ascontiguousarray(np.asarray(Wk, np.float32)[fs, :].T).astype(ml_dtypes.bfloat16),
                "wv": np.ascontiguousarray(np.asarray(Wv, np.float32)[fs, :].T).astype(ml_dtypes.bfloat16),
                "wo": np.ascontiguousarray(
                    np.asarray(Wo, np.float32)[:, fs].T
                ).astype(ml_dtypes.bfloat16),
                "bqk": np.stack(
                    [
                        np.asarray(bq, np.float32)[fs].reshape(FT, 128),
                        np.asarray(bk, np.float32)[fs].reshape(FT, 128),
                    ]
                ),
                "bv": np.ascontiguousarray(np.asarray(bv, np.float32)[fs]),
            }
        )
    return in_maps


def _enable_ldw_opt():
    """Flip walrus's --enable-ldw-opt to true: it hides LDWEIGHTS behind
    matmul streaming (background weight buffer), which is worth ~90ns on
    every matmul here."""
    import concourse.bass_utils as _bu

    if getattr(_bu, "_ldw_patched", False):
        return
    _orig = _bu.run_command

    def _patched(argv, **kwargs):
        argv = [
            a.replace("--enable-ldw-opt=false", "--enable-ldw-opt=true")
            if isinstance(a, str)
            else a
            for a in argv
        ]
        return _orig(argv, **kwargs)

    _bu.run_command = _patched
    _bu._ldw_patched = True


def kernel(query, key, value, Wq, bq, Wk, bk, Wv, bv, Wo, bo, _trace=False):
    nc = build_program()
    _legalize_waits(nc)
    in_maps = make_in_maps(query, key, value, Wq, bq, Wk, bk, Wv, bv, Wo, bo)
    try:
        res = run_bass_kernel_spmd(
            nc, in_maps, core_ids=list(range(NCORES)), trace=_trace
        )
    except ModuleNotFoundError:
        res = run_bass_kernel_spmd(nc, in_maps, core_ids=list(range(NCORES)))
    full = np.empty((B, L, D), np.float32)
    bo32 = np.asarray(bo, np.float32)
    for b in range(B):
        full[b] = res.results[2 * b]["out"] + res.results[2 * b + 1]["out"] + bo32
    if _trace:
        kernel._last_trace = res
    return full

